# revision 4
# baseline (speedup 1.0000x reference)
"""Trainium2 Bass kernel for nn_CheiralityLayer (cheirality loss) — v2.

Reference (per batch element b):
  gray = mean(img_pair[b, :3], axis=0)                       # [H, W]
  gx[h,w] = gray[h,w+1] - gray[h,w-1]   (zero padded)
  gy[h,w] = gray[h+1,w] - gray[h-1,w]   (zero padded)
  n = sqrt(gx^2 + gy^2 + 1e-8)
  P = gx*(x*V2 - V0) + gy*(y*V2 - V1)
  R = gx*(W0*x*y - W1*(x^2+1) + W2*y) + gy*(W0*(y^2+1) - W1*x*y - W2*x)
  rho = (P/n) * (nf0 + nf1 - R/n)
  out = mean(gelu(-rho))   (exact erf gelu)

v2 strategy (data parallel: 2 images/core, 4 row-bands of 120 rows each):
- gray is computed UNSCALED (c0+c1+c2, stationary entries 1.0); since grad
  dirs are normalized the 3x scale cancels exactly when eps -> 9e-8.
- gray (row-select) and gy (row stencil) via PE matmuls off the raw img
  rows with host-built banded bf16 stationaries (entries +-1, exact).
- gx via one 16-bit DVE column-shift subtract on an fp16 zero-padded
  graypad (partition start 0 - HW requires starts in {0,32,64,96}).
- squares + n2 on the otherwise idle Pool engine; 1/n2 fp32 on DVE;
  rinv = sqrt on ACT -> bf16.
- gx,gy normalized (bf16) BEFORE the P/I1/v contractions so PSUM holds
  P/n and v = nfs - R/n directly:
    P/n  = V2*xgxn - V0*gxn + (V2 y - V1) gyn
    I1   = -W1*xgxn + (W0 y) gxn + (-W1 y - W2) gyn
    tXn  = (-I1)*x      (ACT drains I1 with scale=-1, DVE multiplies by x)
    v    = (nf0+nf1) + tXn + (W1 - W2 y) gxn - W0(y^2+1) gyn
  and rho = (P/n) * v via one DVE multiply off the drained P.
- all diag/scalar stationaries are built ON-CHIP (iota mask times
  per-partition value column) from a tiny DMA'd table: the baseline's 4MB
  constants DMA (~12us) becomes ~100KB.
- per-tile erf-GELU(+accum) on ACT hides the reduction tail.
"""

import numpy as np

B, C, H, W = 16, 6, 480, 640
NCORES = 8
BPC = B // NCORES          # images per core
NPOS = 4                   # row bands per image
NT = BPC * NPOS            # tiles per core
TH = 120                   # output rows per band
LR = 124                   # loaded img rows per band (stencil halo)
RS = [0, 118, 238, 356]    # first loaded img row per band (clamped)
NSPLIT = [(0, 320), (320, 640)]
EPS = 9e-8                 # 9x reference eps (gray unscaled by 3)

NDIAG = BPC * NPOS * 5     # y-affine diag value columns
NSID = BPC * 3             # scalar-identity value columns
NVAL = NDIAG + NSID

F_B = 2 * NPOS * TH        # gmat1 + dmat2 columns (bf16 tensor)

_CACHE = {}


def _build_program(check_mode=False):
    """check_mode: skip the gelu+reduce tail (CoreSim lacks Gelu) and DMA
    per-pixel rho out as [TH, NT*W] f32 for numerical validation."""
    import concourse.bacc as bacc
    import concourse.tile as tile
    import concourse.mybir as mybir
    from contextlib import ExitStack

    f32 = mybir.dt.float32
    f32r = mybir.dt.float32r
    bf16 = mybir.dt.bfloat16
    f16 = mybir.dt.float16
    i16 = mybir.dt.int16
    AF = mybir.ActivationFunctionType
    OP = mybir.AluOpType

    nc = bacc.Bacc(
        "TRN2", target_bir_lowering=False, debug=False, enable_asserts=False
    )

    img_d = nc.dram_tensor("img3", [BPC, 3, H, W], f32r, kind="ExternalInput").ap()
    nf_d = nc.dram_tensor("nf", [BPC, 2, H, W], f32r, kind="ExternalInput").ap()
    cstb_d = nc.dram_tensor("cstb", [LR, F_B], f32r, kind="ExternalInput").ap()
    cstv_d = nc.dram_tensor("cstv", [128, NVAL], f32, kind="ExternalInput").ap()
    csth_d = nc.dram_tensor("csth", [128, W], f32, kind="ExternalInput").ap()
    if check_mode:
        rho_d = nc.dram_tensor(
            "rho_dbg", [TH, NT * W], f32, kind="ExternalOutput"
        ).ap()
    out_d = nc.dram_tensor("out", [1, 1], f32, kind="ExternalOutput").ap()

    def half(x):
        """[P, 640] AP -> [P, 2, 320] view matching psum halves."""
        return x.rearrange("p (b c) -> p b c", b=2)

    with tile.TileContext(nc) as tc, ExitStack() as ctx:
        consts = ctx.enter_context(tc.tile_pool(name="consts", bufs=1))
        imgp = ctx.enter_context(tc.tile_pool(name="imgp", bufs=3))
        nfp = ctx.enter_context(tc.tile_pool(name="nfp", bufs=2))
        work = ctx.enter_context(tc.tile_pool(name="work", bufs=2))
        psum = ctx.enter_context(tc.tile_pool(name="psum", bufs=1, space="PSUM"))

        cstb = consts.tile([LR, F_B], f32r)
        nc.sync.dma_start(cstb, cstb_d)
        cstv = consts.tile([128, NVAL], f32)
        nc.sync.dma_start(cstv, cstv_d)
        csth = consts.tile([128, W], f32)
        nc.sync.dma_start(csth, csth_d)
        X16 = consts.tile([128, W], f16)
        nc.vector.tensor_copy(X16, csth)
        X = X16[0:TH, :]

        def gmat(p):
            return cstb[0:LR, p * TH : (p + 1) * TH]

        def dmat(p):
            return cstb[0:LR, (NPOS + p) * TH : (NPOS + p + 1) * TH]

        # identity mask: iota(i - p) == 0
        io16 = consts.tile([TH, TH], i16)
        nc.gpsimd.iota(io16, [[1, TH]], base=0, channel_multiplier=-1)
        mask = consts.tile([TH, TH], bf16)
        nc.vector.tensor_scalar(mask, io16, 0, None, OP.is_equal)

        # on-chip diag/sid stationaries from per-partition value columns
        dgt = consts.tile([TH, NDIAG * TH], bf16)
        for col in range(NDIAG):
            nc.vector.tensor_scalar_mul(
                dgt[:, col * TH : (col + 1) * TH], mask, cstv[0:TH, col : col + 1]
            )
        sidt = consts.tile([TH, NSID * TH], bf16)
        for s in range(NSID):
            nc.vector.tensor_scalar_mul(
                sidt[:, s * TH : (s + 1) * TH],
                mask,
                cstv[0:TH, NDIAG + s : NDIAG + s + 1],
            )

        def dg(i, p, k):
            col = ((i * NPOS) + p) * 5 + k
            return dgt[0:TH, col * TH : (col + 1) * TH]

        def sid(i, k):
            s = i * 3 + k
            return sidt[0:TH, s * TH : (s + 1) * TH]

        acc = consts.tile([128, NT], f32)
        nc.vector.memset(acc, 0.0)
        ones_t = consts.tile([128, 1], f32)
        nc.vector.memset(ones_t, 1.0)

        rho_all = consts.tile([128, NT * W], bf16)

        def front(t):
            i, p = divmod(t, NPOS)
            imgt = imgp.tile([LR, 3, W], f32r, tag="imgt")
            nc.sync.dma_start(
                imgt,
                img_d[i, :, RS[p] : RS[p] + LR, :].rearrange("c h w -> h c w"),
            )
            nft = nfp.tile([TH, 2, W], f32r, tag="nft")
            nc.sync.dma_start(
                nft,
                nf_d[i, :, TH * p : TH * (p + 1), :].rearrange("c h w -> h c w"),
            )

            # gray (unscaled channel sum, row-select) on PE
            gray_ps = psum.tile([TH, 2, 512], f32, tag="gray")
            for c3 in range(3):
                for b, (n0, n1) in enumerate(NSPLIT):
                    nc.tensor.matmul(
                        gray_ps[:, b, 0:320],
                        gmat(p),
                        imgt[:, c3, n0:n1],
                        start=(c3 == 0),
                        stop=(c3 == 2),
                    )
            graypad = work.tile([TH, W + 2], f16, tag="graypad")
            nc.gpsimd.memset(graypad[:, 0:1], 0.0)
            nc.gpsimd.memset(graypad[:, W + 1 : W + 2], 0.0)
            nc.scalar.copy(half(graypad[:, 1 : W + 1]), gray_ps[:, :, 0:320])

            # gy (row stencil) on PE off the raw img rows
            gy_ps = psum.tile([TH, 2, 512], f32, tag="gy")
            for c3 in range(3):
                for b, (n0, n1) in enumerate(NSPLIT):
                    nc.tensor.matmul(
                        gy_ps[:, b, 0:320],
                        dmat(p),
                        imgt[:, c3, n0:n1],
                        start=(c3 == 0),
                        stop=(c3 == 2),
                    )
            gy = work.tile([TH, W], f16, tag="gy")
            nc.scalar.copy(half(gy), gy_ps[:, :, 0:320])
            gy2 = work.tile([TH, W], f32, tag="gy2")
            nc.gpsimd.tensor_mul(gy2, gy, gy)

            # gx: 16-bit column-shift subtract (partition start 0)
            gx = work.tile([TH, W], f16, tag="gx")
            nc.vector.tensor_sub(gx, graypad[:, 2 : W + 2], graypad[:, 0:W])
            gx2 = work.tile([TH, W], f32, tag="gx2")
            nc.gpsimd.tensor_mul(gx2, gx, gx)

            n2 = work.tile([TH, W], f32, tag="n2")
            nc.vector.scalar_tensor_tensor(n2, gx2, EPS, gy2, OP.add, OP.add)
            inv2 = work.tile([TH, W], f32, tag="inv2")
            nc.vector.reciprocal_approx_fast(out=inv2, in_=n2)
            rinv = work.tile([TH, W], bf16, tag="rinv")
            nc.scalar.sqrt(rinv, inv2)
            return (t, i, p, nft, gx, gy, rinv)

        def back1(st):
            t, i, p, nft, gx, gy, rinv = st
            gxn = work.tile([TH, W], bf16, tag="gxn")
            nc.vector.tensor_mul(gxn, gx, rinv)
            gyn = work.tile([TH, W], bf16, tag="gyn")
            nc.vector.tensor_mul(gyn, gy, rinv)
            xgxn = work.tile([TH, W], bf16, tag="xgxn")
            nc.vector.tensor_mul(xgxn, gxn, X)
            nfs = work.tile([TH, W], bf16, tag="nfs")
            nc.gpsimd.tensor_add(nfs, nft[:, 0, :], nft[:, 1, :])

            # P/n = V2*xgxn - V0*gxn + (V2 y - V1) gyn
            P_ps = psum.tile([TH, 2, 512], f32, tag="pv")
            psrc = [(sid(i, 0), xgxn), (sid(i, 1), gxn), (dg(i, p, 0), gyn)]
            for k, (m, src) in enumerate(psrc):
                for b, (n0, n1) in enumerate(NSPLIT):
                    nc.tensor.matmul(
                        P_ps[:, b, 0:320],
                        m,
                        src[:, n0:n1],
                        start=(k == 0),
                        stop=(k == len(psrc) - 1),
                    )

            # I1 = -W1*xgxn + (W0 y) gxn + (-W1 y - W2) gyn
            I1_ps = psum.tile([TH, 2, 512], f32, tag="i1")
            isrc = [(sid(i, 2), xgxn), (dg(i, p, 1), gxn), (dg(i, p, 2), gyn)]
            for k, (m, src) in enumerate(isrc):
                for b, (n0, n1) in enumerate(NSPLIT):
                    nc.tensor.matmul(
                        I1_ps[:, b, 0:320],
                        m,
                        src[:, n0:n1],
                        start=(k == 0),
                        stop=(k == len(isrc) - 1),
                    )
            return (t, i, p, P_ps, I1_ps, gxn, gyn, nfs)

        def back2(st2):
            t, i, p, P_ps, I1_ps, gxn, gyn, nfs = st2
            I1b = work.tile([TH, W], bf16, tag="i1b")
            nc.scalar.activation(half(I1b), I1_ps[:, :, 0:320], AF.Copy, scale=-1.0)
            tXn = work.tile([TH, W], bf16, tag="txn")
            nc.vector.tensor_mul(tXn, I1b, X)

            Pb = work.tile([TH, W], bf16, tag="pb")
            nc.scalar.copy(half(Pb), P_ps[:, :, 0:320])

            # v = nfs + tXn + (W1 - W2 y) gxn - W0(y^2+1) gyn (reuses P banks)
            v_ps = psum.tile([TH, 2, 512], f32, tag="pv")
            vsrc = [
                (mask, nfs),
                (mask, tXn),
                (dg(i, p, 3), gxn),
                (dg(i, p, 4), gyn),
            ]
            for k, (m, src) in enumerate(vsrc):
                for b, (n0, n1) in enumerate(NSPLIT):
                    nc.tensor.matmul(
                        v_ps[:, b, 0:320],
                        m,
                        src[:, n0:n1],
                        start=(k == 0),
                        stop=(k == len(vsrc) - 1),
                    )

            rho = rho_all[0:TH, t * W : (t + 1) * W]
            nc.vector.tensor_mul(half(rho), half(Pb), v_ps[:, :, 0:320])
            if check_mode:
                rho32 = work.tile([TH, W], f32, tag="rho32")
                nc.vector.tensor_copy(rho32, rho)
                nc.sync.dma_start(rho_d[:, t * W : (t + 1) * W], rho32)

        st = front(0)
        for t in range(NT):
            st2 = back1(st)
            nst = front(t + 1) if t + 1 < NT else None
            back2(st2)
            st = nst

        gelu_out = consts.tile([128, NT * W], bf16)
        if not check_mode:
            nc.scalar.activation(
                gelu_out[0:TH, :],
                rho_all[0:TH, :],
                AF.Gelu,
                scale=-1.0,
                accum_out=acc[0:TH, 0:1],
            )
        accs = consts.tile([128, 1], f32)
        nc.vector.reduce_sum(
            accs[0:TH, :], acc[0:TH, 0:NT], axis=mybir.AxisListType.X
        )
        out_ps = psum.tile([1, 1], f32, tag="gray")
        nc.tensor.matmul(
            out_ps, accs[0:TH, :], ones_t[0:TH, :], start=True, stop=True
        )
        res = consts.tile([1, 1], f32)
        nc.scalar.copy(res, out_ps)
        nc.sync.dma_start(out_d, res)

    nc.compile()
    return nc


def _host_constants(pose_np):
    """Per-core host-built constants.

    Returns (cstb, cstv_list, csth): cstb/csth shared, cstv per core."""
    # gmat1: img row (RS[p]+k) -> gray row (120p + j), entries 1.0
    # dmat2: gy[j] = gray[120p+j+1] - gray[120p+j-1] (zero padded rows)
    gmat1 = np.zeros((LR, NPOS, TH), np.float32)
    dmat2 = np.zeros((LR, NPOS, TH), np.float32)
    for p in range(NPOS):
        for j in range(TH):
            row = TH * p + j
            gmat1[row - RS[p], p, j] = 1.0
            if row + 1 <= H - 1:
                dmat2[row + 1 - RS[p], p, j] += 1.0
            if row - 1 >= 0:
                dmat2[row - 1 - RS[p], p, j] -= 1.0
    cstb = np.concatenate(
        [gmat1.reshape(LR, -1), dmat2.reshape(LR, -1)], axis=1
    )

    csth = np.broadcast_to(np.arange(W, dtype=np.float32), (128, W)).copy()

    cstv_list = []
    for core in range(NCORES):
        vals = np.zeros((128, NVAL), np.float32)
        for i in range(BPC):
            b = core * BPC + i
            V0, V1, V2, W0, W1, W2 = [float(x) for x in pose_np[b]]
            for p in range(NPOS):
                yv = (TH * p + np.arange(TH)).astype(np.float32)
                base = ((i * NPOS) + p) * 5
                vals[0:TH, base + 0] = V2 * yv - V1
                vals[0:TH, base + 1] = W0 * yv
                vals[0:TH, base + 2] = -W1 * yv - W2
                vals[0:TH, base + 3] = W1 - W2 * yv
                vals[0:TH, base + 4] = -W0 * (yv * yv + 1.0)
            vals[0:TH, NDIAG + i * 3 + 0] = V2
            vals[0:TH, NDIAG + i * 3 + 1] = -V0
            vals[0:TH, NDIAG + i * 3 + 2] = -W1
        cstv_list.append(vals)
    return cstb, cstv_list, csth


def kernel(img_pair, pose, normal_flow):
    from concourse.bass_utils import run_bass_kernel_spmd

    img_pair = np.asarray(img_pair, dtype=np.float32)
    pose = np.asarray(pose, dtype=np.float32)
    normal_flow = np.asarray(normal_flow, dtype=np.float32)

    if "nc" not in _CACHE:
        _CACHE["nc"] = _build_program()
    nc = _CACHE["nc"]

    cstb, cstv_list, csth = _host_constants(pose)
    in_maps = []
    for core in range(NCORES):
        b0 = core * BPC
        in_maps.append(
            {
                "img3": np.ascontiguousarray(img_pair[b0 : b0 + BPC, :3]),
                "nf": np.ascontiguousarray(normal_flow[b0 : b0 + BPC]),
                "cstb": cstb,
                "cstv": cstv_list[core],
                "csth": csth,
            }
        )

    _CACHE["in_maps"] = in_maps
    res = run_bass_kernel_spmd(nc, in_maps, core_ids=list(range(NCORES)))
    total = np.float64(0.0)
    for r in res.results:
        total += np.float64(r["out"][0, 0])
    out = np.float32(total / (B * H * W))
    return np.asarray(out, dtype=np.float32)
